# revision 1
# baseline (speedup 1.0000x reference)
"""DiagBlockAttention Trainium2 kernel.

Full inputs Q,K,V [16, 8192, 256] fp32. Block-diagonal causal attention with
block size 128; returns (out [16,8192,256], A_flat [16,8192,128]).

Sharding: data-parallel over batch — 8 cores x 2 batches each. Each core
processes its 128 independent [128 x 256] blocks; no cross-core comms.

Per-block dataflow on one NeuronCore:
  - load Qb, Kb, Vb [128t, 256e] naturally (HWDGE DMA)
  - PE: transpose Qb, Kb into [e, t] chunks via identity matmuls (PSUM)
  - DVE: copy transposed Qt|Kt PSUM -> SBUF
  - PE: Zt[k,q] = sum_e Kt[e,k] Qt[e,q] (2 accumulating matmuls)
  - ACT: E = exp(Zt / sqrt(e)) PSUM -> SBUF
  - GPSIMD: affine_select zeroes E where k > q (block-causal mask)
  - PE: [O_un | s] = E^T-contract: matmul(lhsT=E, rhs=Vb) and rhs=ones
  - DVE: r = 1/s ; O = O_un * r (per-partition scalar broadcast)
  - PE: A_un[q,k] = E.T via identity matmul; ACT: A = A_un * r
  - store O, A
"""

import numpy as np

BS = 128
T = 8192
EMB = 256
B = 16
N_CORES = 8
B_LOCAL = B // N_CORES

_CACHE = {}


def _build(b_local=B_LOCAL, t=T):
    import concourse.bass as bass  # noqa: F401
    import concourse.mybir as mybir
    import concourse.tile as tile
    from concourse import bacc
    from concourse.masks import make_identity

    f32 = mybir.dt.float32
    nblk = b_local * t // BS
    scale = 1.0 / float(np.sqrt(EMB))

    nc = bacc.Bacc("TRN2", target_bir_lowering=False, debug=False,
                   num_devices=N_CORES)
    Q = nc.dram_tensor("Q", [b_local, t, EMB], f32, kind="ExternalInput")
    K = nc.dram_tensor("K", [b_local, t, EMB], f32, kind="ExternalInput")
    V = nc.dram_tensor("V", [b_local, t, EMB], f32, kind="ExternalInput")
    O = nc.dram_tensor("O", [b_local, t, EMB], f32, kind="ExternalOutput")
    A = nc.dram_tensor("A", [b_local, t, BS], f32, kind="ExternalOutput")

    Qv = Q.ap().flatten_outer_dims().rearrange("(n p) e -> n p e", p=BS)
    Kv = K.ap().flatten_outer_dims().rearrange("(n p) e -> n p e", p=BS)
    Vv = V.ap().flatten_outer_dims().rearrange("(n p) e -> n p e", p=BS)
    Ov = O.ap().flatten_outer_dims().rearrange("(n p) e -> n p e", p=BS)
    Av = A.ap().flatten_outer_dims().rearrange("(n p) k -> n p k", p=BS)

    EXP = mybir.ActivationFunctionType.Exp
    CPY = mybir.ActivationFunctionType.Copy

    with tile.TileContext(nc) as tc:
        with (
            tc.tile_pool(name="const", bufs=1) as cpool,
            tc.tile_pool(name="io", bufs=3) as io,
            tc.tile_pool(name="work", bufs=3) as work,
            tc.tile_pool(name="ps", bufs=2, space="PSUM") as ps,
        ):
            ident = cpool.tile([BS, BS], f32)
            make_identity(nc, ident)
            ones = cpool.tile([BS, 1], f32)
            nc.vector.memset(ones, 1.0)

            for i in range(nblk):
                qb = io.tile([BS, EMB], f32, tag="qb")
                nc.sync.dma_start(out=qb, in_=Qv[i])
                kb = io.tile([BS, EMB], f32, tag="kb")
                nc.sync.dma_start(out=kb, in_=Kv[i])
                vb = io.tile([BS, EMB], f32, tag="vb")
                nc.sync.dma_start(out=vb, in_=Vv[i])

                # Transpose Q,K 128x128 chunks: tp[:, c] = chunk.T
                tp = ps.tile([BS, 4 * BS], f32, tag="tp")
                nc.tensor.matmul(tp[:, 0:128], lhsT=qb[:, 0:128], rhs=ident,
                                 start=True, stop=True)
                nc.tensor.matmul(tp[:, 128:256], lhsT=qb[:, 128:256],
                                 rhs=ident, start=True, stop=True)
                nc.tensor.matmul(tp[:, 256:384], lhsT=kb[:, 0:128], rhs=ident,
                                 start=True, stop=True)
                nc.tensor.matmul(tp[:, 384:512], lhsT=kb[:, 128:256],
                                 rhs=ident, start=True, stop=True)
                qtkt = work.tile([BS, 4 * BS], f32, tag="qtkt")
                nc.vector.tensor_copy(qtkt, tp)

                # Zt[k, q] accumulated over the two e-chunks
                zt = ps.tile([BS, BS], f32, tag="zt")
                nc.tensor.matmul(zt, lhsT=qtkt[:, 256:384],
                                 rhs=qtkt[:, 0:128], start=True, stop=False)
                nc.tensor.matmul(zt, lhsT=qtkt[:, 384:512],
                                 rhs=qtkt[:, 128:256], start=False, stop=True)

                eraw = work.tile([BS, BS], f32, tag="eraw")
                nc.scalar.activation(eraw, zt, EXP, scale=scale)
                # Block-causal mask: keep E[k, q] where q - k >= 0
                emask = work.tile([BS, BS], f32, tag="emask")
                nc.gpsimd.affine_select(
                    out=emask, in_=eraw,
                    compare_op=mybir.AluOpType.is_ge, fill=0.0,
                    base=0, channel_multiplier=-1, pattern=[[1, BS]],
                )

                # O_un[q, e] and s[q] in one PSUM tile
                op = ps.tile([BS, EMB + 1], f32, tag="op")
                nc.tensor.matmul(op[:, 0:EMB], lhsT=emask, rhs=vb,
                                 start=True, stop=True)
                nc.tensor.matmul(op[:, EMB:EMB + 1], lhsT=emask, rhs=ones,
                                 start=True, stop=True)
                rs = work.tile([BS, 1], f32, tag="rs")
                nc.vector.reciprocal(rs, op[:, EMB:EMB + 1])
                osb = work.tile([BS, EMB], f32, tag="osb")
                nc.vector.tensor_scalar_mul(osb, op[:, 0:EMB], rs)

                # A_un[q, k] = E.T, then normalize rows by r
                atp = ps.tile([BS, BS], f32, tag="atp")
                nc.tensor.matmul(atp, lhsT=emask, rhs=ident,
                                 start=True, stop=True)
                asb = work.tile([BS, BS], f32, tag="asb")
                nc.scalar.activation(asb, atp, CPY, scale=rs)

                nc.sync.dma_start(out=Ov[i], in_=osb)
                nc.sync.dma_start(out=Av[i], in_=asb)

    nc.compile()
    return nc


def _get_nc(b_local=B_LOCAL, t=T):
    key = (b_local, t)
    if key not in _CACHE:
        _CACHE[key] = _build(b_local, t)
    return _CACHE[key]


def kernel(Q, K, V):
    from concourse.bass_utils import run_bass_kernel_spmd

    Q = np.asarray(Q)
    K = np.asarray(K)
    V = np.asarray(V)
    b, t, e = Q.shape
    assert (b, t, e) == (B, T, EMB)

    nc = _get_nc()
    core_ids = list(range(N_CORES))
    in_maps = [
        {
            "Q": np.ascontiguousarray(Q[i * B_LOCAL:(i + 1) * B_LOCAL]),
            "K": np.ascontiguousarray(K[i * B_LOCAL:(i + 1) * B_LOCAL]),
            "V": np.ascontiguousarray(V[i * B_LOCAL:(i + 1) * B_LOCAL]),
        }
        for i in core_ids
    ]
    res = run_bass_kernel_spmd(nc, in_maps, core_ids)
    out = np.concatenate([res.results[i]["O"] for i in core_ids], axis=0)
    a_flat = np.concatenate([res.results[i]["A"] for i in core_ids], axis=0)
    return out, a_flat


# revision 21
# speedup vs baseline: 95.4297x; 95.4297x over previous
"""DiagBlockAttention Trainium2 kernel.

Full inputs Q,K,V [16, 8192, 256] fp32. Block-diagonal causal attention with
block size 128; returns (out [16,8192,256], A_flat [16,8192,128]).

Sharding: data-parallel over batch — 8 cores x 2 batches each. Each core
processes its 128 independent [128 x 256] blocks; no cross-core comms.

Per-core structure: blocks are processed in groups of G=8 so each DMA moves
~1MB (amortizes the ~0.6us/DMA HWDGE descriptor-generation cost; with
per-block DMAs the kernel is HWDGE-bound).

Per-block dataflow:
  - PE transpose-mode: Qb,Kb [128t x 256e] -> [e, t] chunks (PSUM, fp32
    transpose mode is 2 cyc/row vs 4 for a regular fp32 matmul)
  - DVE copies Qt|Kt PSUM -> SBUF
  - PE: Zt[k,q] = sum_e Kt[e,k] Qt[e,q], 2 accumulating fp32 matmuls
  - ACT: E = exp(Zt/sqrt(e)) PSUM -> SBUF;  GPSIMD: zero E where k > q
  - PE: O_un[q,:] = E.T V (float32r, N=256 -> 1 cyc/row), s = E.T ones
  - DVE: r = 1/s; O = O_un * r (per-partition broadcast) into the G-block
    output supertile
  - PE transpose-mode: A_un[q,k] = E.T; ACT: A = A_un * r into supertile
"""

import numpy as np

BS = 128
T = 8192
EMB = 256
B = 16
N_CORES = 8
B_LOCAL = B // N_CORES
G = 8  # blocks per DMA group

_CACHE = {}


def _build(b_local=B_LOCAL, t=T, g=G, repeat=1):
    import concourse.bass as bass  # noqa: F401
    import concourse.mybir as mybir
    import concourse.tile as tile
    from concourse import bacc
    from concourse.masks import make_identity

    f32 = mybir.dt.float32
    nblk = b_local * t // BS
    g = min(g, nblk)
    # group sizes: big groups for DMA efficiency, tapered tail so the
    # final stores don't wait on a whole 8-block compute chain
    gsizes = [g] * (nblk // g)
    assert sum(gsizes) == nblk
    scale = 1.0 / float(np.sqrt(EMB))

    nc = bacc.Bacc("TRN2", target_bir_lowering=False, debug=False,
                   num_devices=N_CORES)
    Q = nc.dram_tensor("Q", [b_local, t, EMB], f32, kind="ExternalInput")
    K = nc.dram_tensor("K", [b_local, t, EMB], f32, kind="ExternalInput")
    V = nc.dram_tensor("V", [b_local, t, EMB], f32, kind="ExternalInput")
    O = nc.dram_tensor("O", [b_local, t, EMB], f32, kind="ExternalOutput")
    A = nc.dram_tensor("A", [b_local, t, BS], f32, kind="ExternalOutput")

    # Block views: [nblk, 128 partitions, row]; groups slice the block axis.
    Qv = Q.ap().flatten_outer_dims().rearrange("(n p) e -> n p e", p=BS)
    Kv = K.ap().flatten_outer_dims().rearrange("(n p) e -> n p e", p=BS)
    Vv = V.ap().flatten_outer_dims().rearrange("(n p) e -> n p e", p=BS)
    Ov = O.ap().flatten_outer_dims().rearrange("(n p) e -> n p e", p=BS)
    Av = A.ap().flatten_outer_dims().rearrange("(n p) k -> n p k", p=BS)

    def grp_view(v, j0, gs):
        # [gs, 128, r] -> [128, gs, r]
        return v[j0:j0 + gs].rearrange("g p e -> p g e")

    EXP = mybir.ActivationFunctionType.Exp
    CPY = mybir.ActivationFunctionType.Copy

    with tile.TileContext(nc) as tc:
        with (
            tc.tile_pool(name="const", bufs=1) as cpool,
            tc.tile_pool(name="io", bufs=3) as io,
            tc.tile_pool(name="work", bufs=4) as work,
            tc.tile_pool(name="ps", bufs=2, space="PSUM") as ps,
        ):
            ident = cpool.tile([BS, BS], f32)
            make_identity(nc, ident)
            ones = cpool.tile([BS, 1], f32)
            nc.vector.memset(ones, 1.0)

            starts = []
            s0 = 0
            for gs in gsizes:
                starts.append((s0, gs))
                s0 += gs
            for rj in range(repeat * len(gsizes)):
                j0, gs = starts[rj % len(starts)]
                q8 = io.tile([BS, g, EMB], f32, tag="q8", name="q8")[:, :gs]
                nc.sync.dma_start(out=q8, in_=grp_view(Qv, j0, gs))
                k8 = io.tile([BS, g, EMB], f32, tag="k8", name="k8")[:, :gs]
                nc.sync.dma_start(out=k8, in_=grp_view(Kv, j0, gs))
                v8 = io.tile([BS, g, EMB], f32, tag="v8", name="v8")[:, :gs]
                nc.sync.dma_start(out=v8, in_=grp_view(Vv, j0, gs))
                o8 = io.tile([BS, g, EMB], f32, tag="o8", name="o8")[:, :gs]
                a8 = io.tile([BS, g, BS], f32, tag="a8", name="a8")[:, :gs]

                for gi in range(gs):
                    qb = q8[:, gi]
                    kb = k8[:, gi]
                    vb = v8[:, gi]

                    # Transpose-mode: tp[:, c*128:(c+1)*128] = chunk.T
                    tp = ps.tile([BS, 4 * BS], f32, tag="tp", bufs=4)
                    nc.tensor.transpose(tp[:, 0:128], qb[:, 0:128], ident)
                    nc.tensor.transpose(tp[:, 128:256], qb[:, 128:256], ident)
                    nc.tensor.transpose(tp[:, 256:384], kb[:, 0:128], ident)
                    nc.tensor.transpose(tp[:, 384:512], kb[:, 128:256], ident)
                    qtkt = work.tile([BS, 4 * BS], f32, tag="qtkt")
                    nc.vector.tensor_copy(qtkt, tp)

                    # One PSUM bank holds Zt (later reused for A_un), O_un
                    # and the softmax denominator: 385 cols * 4B <= 2KB.
                    pz = ps.tile([BS, 385], f32, tag="pz", bufs=4)
                    zt = pz[:, 0:BS]
                    nc.tensor.matmul(zt, lhsT=qtkt[:, 256:384],
                                     rhs=qtkt[:, 0:128],
                                     start=True, stop=False)
                    nc.tensor.matmul(zt, lhsT=qtkt[:, 384:512],
                                     rhs=qtkt[:, 128:256],
                                     start=False, stop=True)

                    eraw = work.tile([BS, BS], f32, tag="eraw")
                    nc.scalar.activation(eraw, zt, EXP, scale=scale)
                    # Block-causal mask: keep E[k, q] where q - k >= 0
                    emask = work.tile([BS, BS], f32, tag="emask")
                    nc.gpsimd.affine_select(
                        out=emask, in_=eraw,
                        compare_op=mybir.AluOpType.is_ge, fill=0.0,
                        base=0, channel_multiplier=-1, pattern=[[1, BS]],
                    )

                    # O_un[q, e] and s[q]
                    op = pz[:, BS:BS + EMB]
                    nc.tensor.matmul(op, lhsT=emask, rhs=vb,
                                     start=True, stop=True)
                    nc.tensor.matmul(pz[:, BS + EMB:BS + EMB + 1], lhsT=emask,
                                     rhs=ones, start=True, stop=True)
                    rs = work.tile([BS, 1], f32, tag="rs")
                    nc.vector.reciprocal(rs, pz[:, BS + EMB:BS + EMB + 1])
                    nc.vector.tensor_scalar_mul(o8[:, gi], op, rs)

                    # A_un[q, k] = E.T into the (now consumed) Zt region
                    atp = pz[:, 0:BS]
                    nc.tensor.transpose(atp, emask, ident)
                    nc.scalar.activation(a8[:, gi], atp, CPY, scale=rs)

                nc.scalar.dma_start(out=grp_view(Ov, j0, gs), in_=o8)
                nc.scalar.dma_start(out=grp_view(Av, j0, gs), in_=a8)

    nc.compile()
    return nc


def _get_nc(b_local=B_LOCAL, t=T):
    key = (b_local, t)
    if key not in _CACHE:
        _CACHE[key] = _build(b_local, t)
    return _CACHE[key]


def kernel(Q, K, V):
    from concourse.bass_utils import run_bass_kernel_spmd

    Q = np.asarray(Q)
    K = np.asarray(K)
    V = np.asarray(V)
    b, t, e = Q.shape
    assert (b, t, e) == (B, T, EMB)

    nc = _get_nc()
    core_ids = list(range(N_CORES))
    in_maps = [
        {
            "Q": np.ascontiguousarray(Q[i * B_LOCAL:(i + 1) * B_LOCAL]),
            "K": np.ascontiguousarray(K[i * B_LOCAL:(i + 1) * B_LOCAL]),
            "V": np.ascontiguousarray(V[i * B_LOCAL:(i + 1) * B_LOCAL]),
        }
        for i in core_ids
    ]
    res = run_bass_kernel_spmd(nc, in_maps, core_ids)
    out = np.concatenate([res.results[i]["O"] for i in core_ids], axis=0)
    a_flat = np.concatenate([res.results[i]["A"] for i in core_ids], axis=0)
    return out, a_flat
